# revision 3
# baseline (speedup 1.0000x reference)
"""TRN2 Bass kernel for nn_MultiHeadAttention (B=2, S=2048, D=1024, H=16, causal).

Sharding: 8 cores = (batch b in {0,1}) x (head-group hg in {0..3}, 4 heads each).
Each core computes Q/K/V projections for its head group (full S), causal
attention for its 4 heads, and a partial output projection against its
256-column slice of w_o.  Host sums the 4 partial Y per batch and adds b_o.

v2 dataflow (per core), balanced across engines:
  PE:   projections (bias folded in via K=1 ones matmuls), QK (head pairs on
        disjoint row groups), PV (col-packed pairs) + 1/l broadcast, out-proj.
  ACT:  exp over wide [128,<=1536] psum score tiles with accum row-sums.
  DVE:  exact row-max reduces, causal mask add (diag block), reciprocal,
        psum->sbuf copies, O^T normalize.
  Pool: l combines + rl stash into E (sbuf-only ops).
  XBAR: ONE batched E^T transpose per row-tile (all 4 heads), bf16.
E is bf16 (fp32-range exponent) so accumulators / 1/l never overflow.
Emission interleaves projections and deferred PV between QK waves so the PE
queue never head-of-line blocks on score-psum reuse.
"""
import numpy as np

B, S, D, H = 2, 2048, 1024, 16
DK = D // H          # 64
HG = 4               # heads per core
DHG = HG * DK        # 256 head dims per core
P = 128
NQT = S // P         # 16 q tiles
KT0 = 512            # projection stream width
SCT = 1536           # score tile width (3 psum banks)
SCALE = float(np.sqrt(DK))  # reference multiplies by sqrt(dk)
MSHIFT = 0.0
NEG = -1.0e30

_FP16 = "float16"


def _build(reps=1, n_cores=8, stash_pool=True, comb_pool=True):
    import concourse.bass as bass
    import concourse.mybir as mybir
    import concourse.tile as tile
    from concourse import bacc

    f32 = mybir.dt.float32
    f16 = getattr(mybir.dt, _FP16)
    bf16 = mybir.dt.bfloat16

    nc = bacc.Bacc("TRN2", target_bir_lowering=False, debug=False,
                   num_devices=n_cores)

    qT = nc.dram_tensor("qT", [D, S], f16, kind="ExternalInput")
    kT = nc.dram_tensor("kT", [D, S], f16, kind="ExternalInput")
    vT = nc.dram_tensor("vT", [D, S], f16, kind="ExternalInput")
    wqT = nc.dram_tensor("wqT", [D, DHG], f16, kind="ExternalInput")
    wkT = nc.dram_tensor("wkT", [D, DHG], f16, kind="ExternalInput")
    wvT = nc.dram_tensor("wvT", [D, DHG], f16, kind="ExternalInput")
    woT = nc.dram_tensor("woT", [DHG, D], f16, kind="ExternalInput")
    bqr = nc.dram_tensor("bqr", [1, DHG], f16, kind="ExternalInput")
    bkr = nc.dram_tensor("bkr", [1, DHG], f16, kind="ExternalInput")
    bvr = nc.dram_tensor("bvr", [1, DHG], f16, kind="ExternalInput")
    mask = nc.dram_tensor("mask", [P, P], f32, kind="ExternalInput")
    y = nc.dram_tensor("y", [S, D], f16, kind="ExternalOutput")

    with tile.TileContext(nc) as tc:
        with (
            tc.tile_pool(name="wpool", bufs=1) as wpool,
            tc.tile_pool(name="persist", bufs=1) as persist,
            tc.tile_pool(name="io", bufs=4) as io,
            tc.tile_pool(name="ew", bufs=2) as ew,
            tc.tile_pool(name="stat", bufs=8) as stat,
            tc.tile_pool(name="ysb", bufs=2) as ysbp,
        ):
            loop_ctx = tc.For_i(0, reps, 1) if reps != 1 else None
            if loop_ctx is not None:
                loop_ctx.__enter__()

            # ---- static weights / consts ----
            wq_sb = wpool.tile([P, D // P, DHG], f16, tag="wq")
            nc.sync.dma_start(wq_sb[:], wqT.rearrange("(c p) n -> p c n", p=P))
            wk_sb = wpool.tile([P, D // P, DHG], f16, tag="wk")
            nc.sync.dma_start(wk_sb[:], wkT.rearrange("(c p) n -> p c n", p=P))
            wv_sb = wpool.tile([P, D // P, DHG], f16, tag="wv")
            nc.sync.dma_start(wv_sb[:], wvT.rearrange("(c p) n -> p c n", p=P))
            wo_sb = wpool.tile([P, DHG // P, D], f16, tag="wo")
            nc.sync.dma_start(wo_sb[:], woT.rearrange("(c p) n -> p c n", p=P))
            bq_sb = wpool.tile([1, DHG], f16, tag="bq")
            nc.sync.dma_start(bq_sb[:], bqr[:])
            bk_sb = wpool.tile([1, DHG], f16, tag="bk")
            nc.sync.dma_start(bk_sb[:], bkr[:])
            bv_sb = wpool.tile([1, DHG], f16, tag="bv")
            nc.sync.dma_start(bv_sb[:], bvr[:])
            mask_sb = wpool.tile([P, P], f32, tag="mask")
            nc.sync.dma_start(mask_sb[:], mask[:])
            ones16 = wpool.tile([1, KT0], f16, tag="ones16")
            nc.vector.memset(ones16[:], 1.0)
            onesbf = wpool.tile([1, P], bf16, tag="onesbf")
            nc.vector.memset(onesbf[:], 1.0)

            # ---- persistent activations ----
            QTs = [persist.tile([P, S], f16, tag=f"QT{m}", name=f"QT{m}")
                   for m in range(2)]
            KTs = [persist.tile([P, S], f16, tag=f"KT{m}", name=f"KT{m}")
                   for m in range(2)]
            Vsb = persist.tile([P, NQT, DHG], bf16, tag="V")
            OTs = [persist.tile([P, S], f16, tag=f"OT{m}", name=f"OT{m}")
                   for m in range(2)]

            with (
                tc.tile_pool(name="sc", bufs=2, space="PSUM") as scp,
                tc.tile_pool(name="scr", bufs=2, space="PSUM") as scrp,
            ):
                # ======== projections for one 512-col stream group ========
                def emit_proj_group(st):
                    sl = slice(st * KT0, (st + 1) * KT0)
                    for (dst, wsb, brow, src) in (
                            (QTs, wq_sb, bq_sb, qT), (KTs, wk_sb, bk_sb, kT)):
                        xt = io.tile([P, D // P, KT0], f16, tag="x", name="xt")
                        nc.sync.dma_start(
                            xt[:], src.rearrange("(c p) s -> p c s", p=P)[:, :, sl])
                        for m in range(2):
                            ps = scp.tile([P, SCT], f32, tag="sct", name="pjp")
                            for c in range(D // P):
                                nc.tensor.matmul(
                                    ps[:, :KT0],
                                    wsb[:, c, m * P:(m + 1) * P],
                                    xt[:, c, :],
                                    start=(c == 0), stop=False,
                                )
                            nc.tensor.matmul(
                                ps[:, :KT0], brow[:, m * P:(m + 1) * P],
                                ones16[:], start=False, stop=True,
                            )
                            nc.vector.tensor_copy(dst[m][:, sl], ps[:, :KT0])
                    # V chunks 4st..4st+3
                    xtv = io.tile([P, D // P, KT0], f16, tag="x", name="xtv")
                    nc.sync.dma_start(
                        xtv[:], vT.rearrange("(c p) s -> p c s", p=P)[:, :, sl])
                    for sub in range(4):
                        ci = 4 * st + sub
                        ps = scp.tile([P, SCT], f32, tag="sct", name="pjv")
                        for c in range(D // P):
                            nc.tensor.matmul(
                                ps[:, :DHG],
                                xtv[:, c, sub * P:(sub + 1) * P],
                                wv_sb[:, c, :],
                                start=(c == 0), stop=False,
                            )
                        nc.tensor.matmul(
                            ps[:, :DHG], ones16[:, :P], bv_sb[:],
                            start=False, stop=True,
                        )
                        nc.vector.tensor_copy(Vsb[:, ci, :], ps[:, :DHG])

                # ======== attention building blocks ========
                def emit_qk_tile(i, pr, ab, t, w):
                    po = ab * DK
                    tw = min(SCT, w - t * SCT)
                    sct = scp.tile([P, SCT], f32, tag="sct", name=f"sct{ab}")
                    for o in range(0, tw, KT0):
                        ow = min(KT0, tw - o)
                        nc.tensor.matmul(
                            sct[:, o:o + ow],
                            QTs[pr][po:po + DK, i * P:(i + 1) * P],
                            KTs[pr][po:po + DK,
                                    t * SCT + o:t * SCT + o + ow],
                            start=True, stop=True,
                        )
                    return (sct, tw)

                def emit_softmax(i, pr, ab, sc_tiles, E):
                    h = 2 * pr + ab
                    w = (i + 1) * P
                    reg = h * (i + 2) * P
                    nt = len(sc_tiles)
                    # causal mask on the diagonal 128 block
                    sct, tw = sc_tiles[-1]
                    nc.vector.tensor_tensor(
                        sct[:, tw - P:tw], sct[:, tw - P:tw],
                        mask_sb[:], mybir.AluOpType.add,
                    )
                    # exact row max (negated)
                    negm = stat.tile([P, 1], f32, tag="negm")
                    if nt == 1:
                        sct, tw = sc_tiles[0]
                        nc.vector.tensor_reduce(
                            negm[:], sct[:, :tw], axis=mybir.AxisListType.X,
                            op=mybir.AluOpType.max, negate=True,
                        )
                    else:
                        msl = stat.tile([P, 2], f32, tag="msl")
                        for t, (sct, tw) in enumerate(sc_tiles):
                            nc.vector.tensor_reduce(
                                msl[:, t:t + 1], sct[:, :tw],
                                axis=mybir.AxisListType.X,
                                op=mybir.AluOpType.max,
                            )
                        nc.vector.tensor_reduce(
                            negm[:], msl[:, :nt], axis=mybir.AxisListType.X,
                            op=mybir.AluOpType.max, negate=True,
                        )
                    # exp + row-sum
                    lsl = stat.tile([P, 2], f32, tag="lsl")
                    for t, (sct, tw) in enumerate(sc_tiles):
                        nc.scalar.activation(
                            E[:, reg + t * SCT:reg + t * SCT + tw],
                            sct[:, :tw],
                            mybir.ActivationFunctionType.Exp,
                            bias=negm[:], scale=1.0,
                            accum_out=lsl[:, t:t + 1],
                        )
                    rl = stat.tile([P, 1], f32, tag="rl")
                    if nt == 1:
                        nc.vector.reciprocal(rl[:], lsl[:, 0:1])
                    else:
                        l = stat.tile([P, 1], f32, tag="l")
                        if comb_pool:
                            nc.gpsimd.tensor_tensor(
                                l[:], lsl[:, 0:1], lsl[:, 1:2],
                                mybir.AluOpType.add)
                        else:
                            nc.vector.tensor_reduce(
                                l[:], lsl[:, :nt], axis=mybir.AxisListType.X,
                                op=mybir.AluOpType.add)
                        nc.vector.reciprocal(rl[:], l[:])
                    # stash 1/l as an extra E column (bf16)
                    eng = nc.gpsimd if stash_pool else nc.vector
                    eng.tensor_copy(E[:, reg + w:reg + w + 1], rl[:])

                def emit_pv_pe(i_, pr_, ET_):
                    pvrb = scrp.tile([P, 2, P], f32, tag="scr", name="pvrb")
                    pv = pvrb[:, 0, :]
                    rb = pvrb[:, 1, :]
                    for c in range(i_ + 1):
                        for ab in range(2):
                            h = 2 * pr_ + ab
                            nc.tensor.matmul(
                                pv[ab * DK:(ab + 1) * DK, :],
                                Vsb[:, c, h * DK:(h + 1) * DK],
                                ET_[:, h * (i_ + 2) + c, :],
                                start=(c == 0), stop=(c == i_),
                                tile_position=(0, ab * DK),
                                skip_group_check=True,
                            )
                    for ab in range(2):
                        h = 2 * pr_ + ab
                        nc.tensor.matmul(
                            rb[ab * DK:(ab + 1) * DK, :],
                            onesbf[:, :DK],
                            ET_[0:1, h * (i_ + 2) + (i_ + 1), :],
                            start=True, stop=True,
                            tile_position=(0, ab * DK),
                            skip_group_check=True,
                        )
                    return (i_, pr_, pv, rb)

                def emit_pv_norm(i_, pr_, pv, rb):
                    rsb = stat.tile([P, P], bf16, tag="rsb")
                    nc.vector.tensor_copy(rsb[:], rb[:])
                    nc.vector.tensor_tensor(
                        OTs[pr_][:, i_ * P:(i_ + 1) * P], pv[:], rsb[:],
                        mybir.AluOpType.mult,
                    )

                # ======== main interleaved loop ========
                pending = None      # (i, ET) awaiting PV
                emit_proj_group(0)
                for i in range(NQT):
                    if i % 4 == 2 and i // 4 + 1 < S // KT0:
                        emit_proj_group(i // 4 + 1)
                    w = (i + 1) * P
                    nt = (w + SCT - 1) // SCT
                    E = ew.tile([P, 4 * (i + 2) * P], bf16, tag="E", name="E")
                    ET = ew.tile([P, 4 * (i + 2), P], bf16, tag="ET", name="ET")
                    parts = []
                    for pr in range(2):
                        sc_ab = [[], []]
                        if nt == 1:
                            # pair-interleaved: one tile per head, 2 bufs
                            sc_ab[0].append(emit_qk_tile(i, pr, 0, 0, w))
                            sc_ab[1].append(emit_qk_tile(i, pr, 1, 0, w))
                            if pending is not None and pending[0] is not None:
                                parts.append(emit_pv_pe(
                                    pending[0], pr, pending[1]))
                            emit_softmax(i, pr, 0, sc_ab[0], E)
                            emit_softmax(i, pr, 1, sc_ab[1], E)
                        else:
                            # head-serial: nt tiles live per head
                            for ab in range(2):
                                for t in range(nt):
                                    sc_ab[ab].append(
                                        emit_qk_tile(i, pr, ab, t, w))
                                if ab == 0 and pending is not None \
                                        and pending[0] is not None:
                                    parts.append(emit_pv_pe(
                                        pending[0], pr, pending[1]))
                                emit_softmax(i, pr, ab, sc_ab[ab], E)
                    for part in parts:
                        emit_pv_norm(*part)
                    # one batched transpose for all 4 heads (+rl columns)
                    nc.sync.dma_start_transpose(
                        ET[:, :4 * (i + 2), :], E[:, :4 * (i + 2) * P])
                    pending = (i, ET)
                if pending is not None:
                    for pr in range(2):
                        emit_pv_norm(*emit_pv_pe(pending[0], pr, pending[1]))

            # ================= output projection (tail) =================
            with tc.tile_pool(name="yp", bufs=4, space="PSUM") as ypp:
                for i in range(NQT):
                    ysb_t = ysbp.tile([P, D], f16, tag="ysb")
                    for nhalf in range(2):
                        yps = ypp.tile([P, KT0], f32, tag="yps")
                        for kc in range(2):
                            nc.tensor.matmul(
                                yps[:],
                                OTs[kc][:, i * P:(i + 1) * P],
                                wo_sb[:, kc, nhalf * KT0:(nhalf + 1) * KT0],
                                start=(kc == 0), stop=(kc == 1),
                            )
                        nc.vector.tensor_copy(
                            ysb_t[:, nhalf * KT0:(nhalf + 1) * KT0], yps[:])
                    nc.sync.dma_start(y[i * P:(i + 1) * P, :], ysb_t[:])

            if loop_ctx is not None:
                loop_ctx.__exit__(None, None, None)

    nc.compile()
    return nc


_NC_CACHE = {}


def _get_nc(reps=1, **kw):
    key = (reps, tuple(sorted(kw.items())))
    if key not in _NC_CACHE:
        _NC_CACHE[key] = _build(reps, **kw)
    return _NC_CACHE[key]


def make_core_inputs(q, k, v, w_q, b_q, w_k, b_k, w_v, b_v, w_o):
    """Host-side shard prep: list of 8 per-core input dicts."""
    f16 = np.dtype(_FP16)
    tri = np.triu(np.full((P, P), NEG, np.float32), k=1)
    in_maps = []
    for c in range(8):
        b, hg = c // 4, c % 4
        sl = slice(hg * DHG, (hg + 1) * DHG)
        wq_s = (w_q[sl] * SCALE).astype(np.float32)
        bq_s = (b_q[sl] * SCALE).astype(np.float32)
        in_maps.append({
            "qT": np.ascontiguousarray(q[b].T).astype(f16),
            "kT": np.ascontiguousarray(k[b].T).astype(f16),
            "vT": np.ascontiguousarray(v[b].T).astype(f16),
            "wqT": np.ascontiguousarray(wq_s.T).astype(f16),
            "wkT": np.ascontiguousarray(w_k[sl].T).astype(f16),
            "wvT": np.ascontiguousarray(w_v[sl].T).astype(f16),
            "woT": np.ascontiguousarray(w_o[:, sl].T).astype(f16),
            "bqr": bq_s.reshape(1, DHG).astype(f16),
            "bkr": b_k[sl].reshape(1, DHG).astype(f16),
            "bvr": b_v[sl].reshape(1, DHG).astype(f16),
            "mask": tri,
            })
    return in_maps


def kernel(k, q, v, mask, w_k, b_k, w_q, b_q, w_v, b_v, w_o, b_o):
    """Full-input entry point. mask is 1 (causal) per the reference."""
    from concourse.bass_utils import run_bass_kernel_spmd

    q = np.asarray(q, np.float32)
    k = np.asarray(k, np.float32)
    v = np.asarray(v, np.float32)
    w_q = np.asarray(w_q, np.float32); b_q = np.asarray(b_q, np.float32)
    w_k = np.asarray(w_k, np.float32); b_k = np.asarray(b_k, np.float32)
    w_v = np.asarray(w_v, np.float32); b_v = np.asarray(b_v, np.float32)
    w_o = np.asarray(w_o, np.float32); b_o = np.asarray(b_o, np.float32)

    nc = _get_nc(1)
    in_maps = make_core_inputs(q, k, v, w_q, b_q, w_k, b_k, w_v, b_v, w_o)
    res = run_bass_kernel_spmd(nc, in_maps, core_ids=list(range(8))).results
    out = np.zeros((B, S, D), np.float32)
    for c in range(8):
        out[c // 4] += res[c]["y"].astype(np.float32)
    out += b_o.astype(np.float32)
    return out
